# revision 59
# baseline (speedup 1.0000x reference)
"""LSTM decoder kernel for Trainium2 (8 NeuronCores, data-parallel over batch).

Reference computation (per batch element b):
    h0 = context_seq[b, -1, :]          # only the LAST timestep is used
    c0 = 0
    for t in range(T):
        gates = h @ (W_ih + W_hh).T + (b_ih + b_hh)     # [4H], order i,f,g,o
        i, f, g, o = split(gates)
        c = sigmoid(f) * c + sigmoid(i) * tanh(g)
        h = sigmoid(o) * tanh(c)
        pred[t] = h @ W_out.T + b_out                   # [O]

Device layout (per core, B=128 batch rows): state kept TRANSPOSED — hT, cT
are [H=128 partitions, B free], so no per-step transposes are needed and
per-partition ACT bias lines up with gate rows. Per-step prediction via a
small matmul (stationary = hT) giving pred [B, 7] naturally, accumulated in
SBUF, one DMA at the end.

End-to-end wall time of kernel() is dominated by the axon tunnel, NOT device
compute (device: ~1.6ms; tunnel: ~24ms/MB up + ~28ms fixed, ~22ms/MB down +
~65ms fixed PER FETCH, ~67ms jit RTT). Measured warm-wall progression
(same link, min over 8 calls):
  f32 everything (variant 7)                      ~708ms
  bf16 compute/transfers + recycled output seeds  ~243ms
  int8 outputs + concurrent shard fetch           ~166ms
  + device-cached inputs, fetch-fused dequant     ~123ms min / ~160ms median
The steady-state floor is ONE execute roundtrip (PJRT pipelines the
output fetches behind execution; upload bytes ride along with the
execute RPC): ~65ms latency + 1.5ms exec + 3.7MB output transfer.
Levers, in order of impact:
  - outputs quantized ON DEVICE to int8 with a per-batch-row scale
    (tensor_reduce abs-max -> reciprocal -> ACT Copy with per-partition
    scale; ACT's f32->int8 conversion rounds to nearest): 3.7MB D2H
    instead of 14.7MB f32. Per-row scales ship as a second tiny output and
    the host dequantizes with exactly 1/scale, so DVE reciprocal error
    cancels. rel err ~8.8e-3 vs the 2e-2 gate (bf16 variant 10: 3.3e-3
    at ~+80ms; plain-f32 variant 7: 7.6e-7 at ~+540ms).
  - every output shard is fetched in a thread pool: D2H fetches have a
    ~65ms FIXED latency each, and concurrent fetches overlap it (a serial
    second 4KB fetch would otherwise cost a full 70ms roundtrip).
  - recurrence/weights in bf16 (PSUM accumulation stays f32), h0/weights
    upload bf16: ~1.3MB H2D instead of 2.6MB.
  - b_out is added ON DEVICE (a [1,B]x[1,4*O] ones-by-bias matmul seeds
    each prediction PSUM accumulation group) so the host does a single
    dequant-multiply pass and no bias add.
  - the donated output buffers (XLA custom-call outputs must be passed in
    as donated parameters) are recycled across calls: the previous call's
    device-resident outputs seed the next call, eliminating a per-call
    output-sized host->device zeros upload (the kernel overwrites every
    element). The first-call seed is device_put with the target sharding
    so later recycled-seed calls hit the same jit executable (no
    recompile on the first warm call).

Variants (per-step time at T=512, 8 cores; >=R101 repeat-slope; numbers
measured same-session unless noted):
  1: single stream, per-gate ACT bias, one gates PSUM bank   (4.2 us/step)
  4: merged sigmoid via K=3 one-hot bias matmul              (5.5 us/step)
  5: variant 4 x two phase-offset streams of B/2             (slow)
  6: variant 1 x two phase-offset streams of B/2             (5.8 us/step)
  7: per-gate ACT bias, gates split into {f,i}/{g,o} PSUM banks so
     sigmoid(f) starts after two matmuls; predictions batched 4 steps per
     PSUM tile/copy                                          (3.4 us/step)
  8: one PSUM bank per gate                                  (4.1 us/step)
 10: variant 7 with bf16 weights/state/outputs (PSUM stays f32) and
     on-device b_out
 11: variant 10 + per-row int8 output quantization           (3.2 us/step)
 13: all-tanh 2-stream rewrite, ~0.73x the step time of 11 measured
     back-to-back (see below)                                <- default

Variant 13 design (the "tanh" style). The per-step critical chain is
latency-bound: every cross-engine hop costs ~100-220ns (sem propagation +
the producer's pipeline-drain before its sem fires) and every ACT visit
carries ~185ns of SBUF-access init on top of ~1 elem/cycle, so the win
comes from MINIMIZING CHAIN VISITS, not engine throughput:
  - sigmoid(z) = 0.5 + 0.5*tanh(z/2) turns all four gate activations
    into ONE ACT tanh over [128, 4*Bs] PSUM. The 1/2s fold into
    pre-scaled weights (host-side, exact powers of two) with states
    tracked as h' = 2h, c' = 2c.
  - per-gate biases can't ride the merged ACT (they vary along the free
    dim), so a [4,H]x[4,4*Bs] one-hot matmul seeds the gates PSUM
    accumulation group; it depends only on the bank, never on h, and
    schedules entirely off the chain.
  - the cell update is three fused scalar_tensor_tensor ops:
        [u|v] = ([tf|ti] + 1) * [c'|tg]      (one STT: operands are made
                                              contiguous by writing c'
                                              into a slot ADJACENT to the
                                              tanh outputs, layout
                                              [c'|tg|tf|ti|to] in a
                                              manually ping-ponged pair
                                              of buffers)
        c'    = 0.5*u + v                    (STT, written into the c'
                                              slot of the next buffer)
        h'    = (to + 1) * tanh(0.5*c')      (ACT tanh + STT)
    Chain per step: PE(5 MM) -> ACT(tanh4) -> DVE(uv) -> DVE(c') ->
    ACT(tanh_c) -> DVE(h') -> PE.  Two ACT visits per step is the
    mathematical minimum for an LSTM (tanh(g) and tanh(c) are serially
    dependent through c).
  - TWO phase-offset streams of B/2: per-instruction FD halves (ACT/DVE
    fixed costs dominate at this size so each instruction barely
    shrinks, but the streams fill each other's hop gaps). 3/4 streams
    simulate no better (hop latencies don't shrink and instruction
    count grows) and measure ~25% worse on HW.
  - step t's prediction matmul is emitted AFTER step t+1's gate matmuls
    (PRED_DEFER): its Ldweights(h'_t) otherwise sits in front of the
    chain-critical gate MMs in the PE queue and delays them each step
    (measured ~160ns/step on HW; the h'->MM trace edge was 337ns vs a
    ~150ns floor before the fix).
  - multi-wait instructions are split into single-wait NoOps (build
    limitation). Wait PLACEMENT matters: a NoOp's wait stalls the whole
    sequencer, while a wait on a real instruction parks in the engine
    wait-queue and lets ready instructions bypass. Same-engine waits
    (usually long satisfied) go on the NoOps; the binding cross-engine
    wait stays on the instruction. NOTE: waits must never be DROPPED —
    engines dispatch out of order from the wait queue, so same-engine
    waits enforce real ordering (dropping them produced rel err 0.88).
"""

import concurrent.futures as _cf
import json

import numpy as np
import ml_dtypes

BF16 = ml_dtypes.bfloat16

# D2H fetches over the axon tunnel have a ~65ms fixed per-fetch latency on
# top of ~45MB/s bandwidth; fetching every output shard concurrently
# overlaps those latencies (measured: 8 parallel shard fetches cost ~one
# latency + total bytes/bandwidth)
_FETCH_POOL = _cf.ThreadPoolExecutor(12)

B_TOTAL = 1024
H = 128
O = 7
N_CORES = 8
B_CORE = B_TOTAL // N_CORES  # 128

VARIANT = 13

ACTS_BUFS = 2
STAGGER = 0  # startup phase-offset self-copies per stream index
PRED_DEFER = True  # emit step t's pred matmul after step t+1's gate MMs
TANH_BF16 = False  # keep t4/uv/th state in bf16 (2x DVE mode; c' precision risk)
_N_STREAMS = {1: 1, 4: 1, 5: 2, 6: 2, 7: 1, 8: 1, 9: 1, 10: 1, 11: 1, 12: 1, 13: 2, 14: 2, 15: 3, 16: 4}
_STYLE = {
    1: "acts",
    4: "biasmm",
    5: "biasmm",
    6: "acts",
    7: "acts2",
    8: "acts4",
    9: "acts2",
    10: "bf16",
    11: "i8",
    12: "tanh",
    13: "tanh",
    14: "tanh",
    15: "tanh",
    16: "tanh",
}
# variants whose gate-tanh is split [g,f,i] + [o] so the cell update can
# start one gate earlier (o is only needed by the h' STT much later)
_TANH_SPLIT_O = (14,)


def _bs_list(n_streams):
    """Per-stream batch sizes (as even as possible) and their offsets."""
    base = B_CORE // n_streams
    rem = B_CORE - base * n_streams
    sizes = [base + (1 if s < rem else 0) for s in range(n_streams)]
    offs = [sum(sizes[:s]) for s in range(n_streams)]
    return sizes, offs
# styles with bf16 recurrence/weights + on-device b_out
_BF_STYLES = ("bf16", "i8", "tanh")


def _pb(style):
    # steps of predictions batched per PSUM tile before one DVE copy
    # (HW A/B at T=512: tanh-style 16 ≥ 8 > 4; fewer DVE copy slots win)
    if style == "tanh":
        return 16
    return 4 if style in ("acts2", "acts4", "bf16", "i8") else 2
# all-tanh styles: sigmoid(x) = 0.5 + 0.5*tanh(x/2) with the 1/2s folded
# into pre-scaled weights and h/c tracked as h'=2h, c'=2c; gate biases
# enter the PSUM accumulation via a [4,H]x[4,4Bs] one-hot seed matmul so
# ONE ACT tanh covers all four gates
_TANH_STYLES = ("tanh",)


def _split_multiwait(bir_bytes: bytes) -> bytes:
    """This walrus build encodes at most ONE sync-wait per instruction.
    Split any multi-wait instruction into single-wait NoOps on the same
    engine (the sequencer executes them in program order, so waiting on
    each semaphore in turn is equivalent to waiting on all of them).

    ALL waits must be kept: engines dispatch out of order from a small
    scoreboard (ready instructions bypass waiting ones), so even waits on
    the instruction's own engine semaphore enforce real ordering.

    Wait placement matters for performance, though: a NoOp's wait stalls
    the sequencer (nothing behind it can dispatch), while the wait kept on
    the real instruction parks in the engine wait-queue and lets later
    ready instructions bypass. So put same-engine waits (usually long
    satisfied — they mostly re-state program order) on the NoOps and keep
    a cross-engine data dependency, likely the binding one, on the
    instruction itself."""
    bir = json.loads(bir_bytes)
    n = 0
    for f in bir.get("functions", []):
        for blk in f.get("blocks", []):
            new = []
            for inst in blk.get("instructions", []):
                si = inst.get("sync_info")
                waits = (si or {}).get("on_wait") or []
                if len(waits) > 1:
                    eng = inst.get("engine")
                    own = f"{eng}_"
                    selfw = [
                        w for w in waits
                        if str(w.get("ant_name") or "").startswith(own)
                    ]
                    crossw = [w for w in waits if w not in selfw]
                    # NoOps first get self-waits, then all but the last
                    # cross-engine wait; the last cross-engine wait (or, if
                    # none, the last self-wait) stays on the instruction
                    ordered = selfw + crossw
                    for w in ordered[:-1]:
                        n += 1
                        nop = {
                            "name": f"WSPLIT-{n}",
                            "engine": inst.get("engine"),
                            "ins": [],
                            "outs": [],
                            "opcode": "NoOp",
                            "sync_info": {"on_update": [], "on_wait": [w]},
                        }
                        if inst.get("debug") is not None:
                            nop["debug"] = inst["debug"]
                        new.append(nop)
                    si["on_wait"] = [ordered[-1]]
                new.append(inst)
            blk["instructions"] = new
    return json.dumps(bir).encode()


_PATCHED = False


def _patch_bass():
    global _PATCHED
    if _PATCHED:
        return
    import concourse.bass as bass

    orig = bass.Bass.to_json_bytes

    def patched(self, *a, **k):
        return _split_multiwait(orig(self, *a, **k))

    bass.Bass.to_json_bytes = patched
    _PATCHED = True


_PROGRAM_CACHE = {}


class _Stream:
    """Per-stream tiles + emit logic for one LSTM step."""

    gp_t2 = False
    split_o = False
    # PSUM is bank-granular per tile buffer (8 banks total); with >2 streams
    # single-buffer the gates/pred psum tiles. The bias seed has ~a full
    # step of slack, so the extra WAR wait stays off the critical chain.
    psum_bufs = 2

    def __init__(self, nc, tc, pools, consts, s, Bs, style, off=None):
        from concourse import mybir

        fp32 = mybir.dt.float32
        bf16 = mybir.dt.bfloat16
        self.nc = nc
        self.s = s
        self.Bs = Bs
        self.off = s * Bs if off is None else off
        self.style = style
        self.consts = consts
        self.state, self.acts, self.psum, self.ppsum, self.outp = pools
        self.h = None  # set by caller
        self.c = None
        out_dt = bf16 if style in _BF_STYLES else fp32
        self.outbuf = self.outp.tile(
            [Bs, consts["T"] * O], out_dt, tag=f"outbuf{s}", name=f"outbuf{s}"
        )
        self.pred_pps = None  # batched pred psum tile
        self.pred_pending = None  # (t, h) of a deferred prediction matmul
        if style == "tanh":
            # manual ping-pong pair of [c' | tg | tf | ti | to] buffers
            t4dt = bf16 if TANH_BF16 else fp32
            self.t4s = [
                self.acts.tile([H, 5 * Bs], t4dt, tag=f"t4{s}_{k}", bufs=1,
                               name=f"t4{s}_{k}")
                for k in range(2)
            ]

    def step(self, t):
        nc = self.nc
        from concourse import mybir

        fp32 = mybir.dt.float32
        bf16 = mybir.dt.bfloat16
        AF = mybir.ActivationFunctionType
        s, Bs = self.s, self.Bs
        C = self.consts
        wt, woutt = C["wt"], C["woutt"]
        bias = C.get("bias")

        if self.style == "tanh":
            # all-tanh recurrence on pre-scaled weights (see _TANH_STYLES).
            # Gate order in PSUM/wt/b4 is (g, f, i, o). The per-step tanh
            # buffer t4s[t%2] is laid out [c' | tg | tf | ti | to] (5 blocks
            # of Bs) so ONE fused STT computes both cell-update products:
            #   [u|v] = ([tf|ti] + 1) * [c'|tg]     (contiguous operands)
            #   c'    = 0.5*u + v                   -> written into the c'
            #                                          slot of t4s[(t+1)%2]
            #   th    = tanh(0.5 * c')
            #   h'    = (to + 1) * th
            add_, mult_ = mybir.AluOpType.add, mybir.AluOpType.mult
            cur = self.t4s[t % 2]
            nxt = self.t4s[(t + 1) % 2]
            gp = self.psum.tile([128, 4 * Bs], fp32, tag=f"g{s}",
                                bufs=self.psum_bufs)
            # bias seed: independent of h, schedulable as soon as the bank
            # frees
            oh = C["onehot4"][:, 4 * self.off : 4 * self.off + 4 * Bs]
            nc.tensor.matmul(gp[:], C["b4"][:], oh,
                             start=True, stop=False, skip_group_check=True)
            for g in range(4):
                nc.tensor.matmul(
                    gp[:, g * Bs : (g + 1) * Bs],
                    wt[:, g * H : (g + 1) * H],
                    self.h[:],
                    start=False,
                    stop=(g == 3),
                    skip_group_check=True,
                )
            # previous step's prediction, emitted AFTER this step's gate
            # matmuls: its Ldweights(h') would otherwise sit in front of the
            # chain-critical gate MMs in the PE queue and delay them by its
            # own dispatch+load each step (its operand h'_{t-1} is long ready
            # by now, so back here it costs nothing)
            if self.pred_pending is not None:
                tp, hp = self.pred_pending
                self._pred(tp, hp)
                self.pred_pending = None
            if self.split_o:
                nc.scalar.activation(cur[:, Bs : 4 * Bs], gp[:, 0 : 3 * Bs], AF.Tanh)
                nc.scalar.activation(
                    cur[:, 4 * Bs : 5 * Bs], gp[:, 3 * Bs : 4 * Bs], AF.Tanh
                )
            else:
                nc.scalar.activation(cur[:, Bs : 5 * Bs], gp[:], AF.Tanh)
            sdt = bf16 if TANH_BF16 else fp32
            uv = self.acts.tile([H, 2 * Bs], sdt, tag=f"uv{s}", name=f"uv{s}")
            nc.vector.scalar_tensor_tensor(
                uv[:], cur[:, 2 * Bs : 4 * Bs], 1.0, cur[:, 0 : 2 * Bs],
                add_, mult_,
            )
            nc.vector.scalar_tensor_tensor(
                nxt[:, 0:Bs], uv[:, 0:Bs], 0.5, uv[:, Bs : 2 * Bs],
                mult_, add_,
            )
            th = self.acts.tile([H, Bs], sdt, tag=f"th{s}", name=f"th{s}")
            nc.scalar.activation(th[:], nxt[:, 0:Bs], AF.Tanh, scale=0.5)
            h_new = self.state.tile([H, Bs], bf16, tag=f"h{s}", name=f"h{s}")
            nc.vector.scalar_tensor_tensor(
                h_new[:], cur[:, 4 * Bs : 5 * Bs], 1.0, th[:], add_, mult_
            )
            self.h = h_new
            if PRED_DEFER:
                self.pred_pending = (t, h_new)
            else:
                self._pred(t)
            return

        if self.style == "biasmm":
            bstack, onehot = C["bstack"], C["onehot"]
            gp = self.psum.tile([128, 4 * Bs], fp32, tag=f"g{s}", bufs=2)
            # bias init for i,f,o cols [0:3Bs) — independent of h, prefetchable
            nc.tensor.matmul(gp[:, 0 : 3 * Bs], bstack[:], onehot[:],
                             start=True, stop=False, skip_group_check=True)
            for g in range(4):  # gate order in wt: i,f,o,g
                nc.tensor.matmul(
                    gp[:, g * Bs : (g + 1) * Bs],
                    wt[:, g * H : (g + 1) * H],
                    self.h[:],
                    start=False,
                    stop=(g == 3),
                    skip_group_check=True,
                )
            ifo = self.acts.tile([H, 3 * Bs], fp32, tag=f"ifo{s}", name=f"ifo{s}")
            nc.scalar.activation(ifo[:], gp[:, 0 : 3 * Bs], AF.Sigmoid)
            g_t = self.acts.tile([H, Bs], fp32, tag=f"gt{s}", name=f"gt{s}")
            nc.scalar.activation(g_t[:], gp[:, 3 * Bs : 4 * Bs], AF.Tanh,
                                 bias=bias[:, 3:4])
            i_s = ifo[:, 0:Bs]
            f_s = ifo[:, Bs : 2 * Bs]
            o_s = ifo[:, 2 * Bs : 3 * Bs]
        elif self.style == "acts4":
            # one PSUM bank per gate: each sigmoid starts right after its own
            # matmul; wt order i,f,g,o; ACT order f,i,g,o
            banks = {}
            for g, tag, bufs in ((1, "bf", 2), (0, "bi", 2), (2, "bg", 1), (3, "bo", 1)):
                pb = self.psum.tile([128, Bs], fp32, tag=f"{tag}{s}", bufs=bufs)
                nc.tensor.matmul(pb[:], wt[:, g * H : (g + 1) * H], self.h[:],
                                 start=True, stop=True)
                banks[g] = pb
            f_t = self.acts.tile([H, Bs], fp32, tag=f"fs{s}", name=f"fs{s}")
            nc.scalar.activation(f_t[:], banks[1][:], AF.Sigmoid, bias=bias[:, 1:2])
            i_t = self.acts.tile([H, Bs], fp32, tag=f"is{s}", name=f"is{s}")
            nc.scalar.activation(i_t[:], banks[0][:], AF.Sigmoid, bias=bias[:, 0:1])
            g_t = self.acts.tile([H, Bs], fp32, tag=f"gt{s}", name=f"gt{s}")
            nc.scalar.activation(g_t[:], banks[2][:], AF.Tanh, bias=bias[:, 2:3])
            o_t = self.acts.tile([H, Bs], fp32, tag=f"os{s}", name=f"os{s}")
            nc.scalar.activation(o_t[:], banks[3][:], AF.Sigmoid, bias=bias[:, 3:4])
            i_s, f_s, g_t, o_s = i_t[:], f_t[:], g_t, o_t[:]
        elif self.style in ("acts2", "bf16", "i8"):
            # per-gate ACT bias, but gates split across TWO PSUM banks
            # ({f,i} and {g,o}) so sigmoid(f) starts after two matmuls
            # instead of four; wt order i,f,g,o
            gfi = self.psum.tile([128, 2 * Bs], fp32, tag=f"gfi{s}", bufs=2)
            ggo = self.psum.tile([128, 2 * Bs], fp32, tag=f"ggo{s}", bufs=1)
            for g, dst, col in ((1, gfi, 0), (0, gfi, 1), (2, ggo, 0), (3, ggo, 1)):
                nc.tensor.matmul(
                    dst[:, col * Bs : (col + 1) * Bs],
                    wt[:, g * H : (g + 1) * H],
                    self.h[:],
                    start=True,
                    stop=True,
                )
            f_t = self.acts.tile([H, Bs], fp32, tag=f"fs{s}", name=f"fs{s}")
            nc.scalar.activation(f_t[:], gfi[:, 0:Bs], AF.Sigmoid, bias=bias[:, 1:2])
            i_t = self.acts.tile([H, Bs], fp32, tag=f"is{s}", name=f"is{s}")
            nc.scalar.activation(i_t[:], gfi[:, Bs : 2 * Bs], AF.Sigmoid, bias=bias[:, 0:1])
            g_t = self.acts.tile([H, Bs], fp32, tag=f"gt{s}", name=f"gt{s}")
            nc.scalar.activation(g_t[:], ggo[:, 0:Bs], AF.Tanh, bias=bias[:, 2:3])
            o_t = self.acts.tile([H, Bs], fp32, tag=f"os{s}", name=f"os{s}")
            nc.scalar.activation(o_t[:], ggo[:, Bs : 2 * Bs], AF.Sigmoid, bias=bias[:, 3:4])
            i_s, f_s, g_t, o_s = i_t[:], f_t[:], g_t, o_t[:]
        else:  # "acts": per-gate ACT with per-partition bias; wt order i,f,g,o
            gp = self.psum.tile([128, 4 * Bs], fp32, tag=f"g{s}", bufs=2)
            for g in (1, 0, 2, 3):  # emit f first: t1 depends on f alone
                nc.tensor.matmul(
                    gp[:, g * Bs : (g + 1) * Bs],
                    wt[:, g * H : (g + 1) * H],
                    self.h[:],
                    start=True,
                    stop=True,
                )
            f_t = self.acts.tile([H, Bs], fp32, tag=f"fs{s}", name=f"fs{s}")
            nc.scalar.activation(f_t[:], gp[:, Bs : 2 * Bs], AF.Sigmoid, bias=bias[:, 1:2])
            i_t = self.acts.tile([H, Bs], fp32, tag=f"is{s}", name=f"is{s}")
            nc.scalar.activation(i_t[:], gp[:, 0:Bs], AF.Sigmoid, bias=bias[:, 0:1])
            g_t = self.acts.tile([H, Bs], fp32, tag=f"gt{s}", name=f"gt{s}")
            nc.scalar.activation(g_t[:], gp[:, 2 * Bs : 3 * Bs], AF.Tanh, bias=bias[:, 2:3])
            o_t = self.acts.tile([H, Bs], fp32, tag=f"os{s}", name=f"os{s}")
            nc.scalar.activation(o_t[:], gp[:, 3 * Bs : 4 * Bs], AF.Sigmoid, bias=bias[:, 3:4])
            i_s, f_s, g_t, o_s = i_t[:], f_t[:], g_t, o_t[:]

        t1 = self.acts.tile([H, Bs], fp32, tag=f"t1{s}", name=f"t1{s}")
        nc.vector.tensor_mul(t1[:], f_s, self.c[:])
        t2 = self.acts.tile([H, Bs], fp32, tag=f"t2{s}", name=f"t2{s}")
        if self.gp_t2:
            nc.gpsimd.tensor_mul(t2[:], i_s, g_t[:])
        else:
            nc.vector.tensor_mul(t2[:], i_s, g_t[:])
        c_new = self.state.tile([H, Bs], fp32, tag=f"c{s}", name=f"c{s}")
        nc.vector.tensor_add(c_new[:], t1[:], t2[:])
        th = self.acts.tile([H, Bs], fp32, tag=f"th{s}", name=f"th{s}")
        nc.scalar.activation(th[:], c_new[:], AF.Tanh)
        h_dt = bf16 if self.style in _BF_STYLES else fp32
        h_new = self.state.tile([H, Bs], h_dt, tag=f"h{s}", name=f"h{s}")
        nc.vector.tensor_mul(h_new[:], o_s, th[:])
        self.h, self.c = h_new, c_new

        self._pred(t)

    def flush_pred(self):
        if self.pred_pending is not None:
            tp, hp = self.pred_pending
            self._pred(tp, hp)
            self.pred_pending = None

    def _pred(self, t, h=None):
        # prediction: out [Bs, O] = h_new.T @ woutt; batch PB steps per PSUM
        # tile + one DVE copy (an accumulation group writing disjoint slots).
        # bf16 style: the group is seeded by a [1,Bs]x[1,PB*O] ones-by-bias
        # matmul that broadcasts b_out into every row/slot, so predictions
        # leave the device with the bias already added.
        nc = self.nc
        from concourse import mybir

        fp32 = mybir.dt.float32
        s, Bs = self.s, self.Bs
        C = self.consts
        woutt = C["woutt"]
        h_new = self.h if h is None else h
        PB = _pb(self.style)
        k = t % PB
        if k == 0:
            self.pred_pps = self.ppsum.tile([Bs, PB * O], fp32, tag=f"pp{s}",
                                             bufs=self.psum_bufs)
            if self.style in _BF_STYLES:
                nc.tensor.matmul(
                    self.pred_pps[:, 0 : PB * O],
                    C["ones"][:, 0:Bs],
                    C["brow"][:],
                    start=True,
                    stop=False,
                    skip_group_check=True,
                )
        first = (k == 0) and self.style not in _BF_STYLES
        nc.tensor.matmul(self.pred_pps[:, k * O : (k + 1) * O], h_new[:], woutt[:],
                         start=first, stop=(k == PB - 1), skip_group_check=True)
        if k == PB - 1 or t == self.consts["T"] - 1:
            nc.vector.tensor_copy(
                self.outbuf[:, (t - k) * O : (t + 1) * O],
                self.pred_pps[:, 0 : (k + 1) * O],
            )


def _build_program(T: int, variant: int = None, repeat: int = 1):
    if variant is None:
        variant = VARIANT
    import concourse.bass as bass
    import concourse.tile as tile
    from concourse import mybir

    _patch_bass()

    fp32 = mybir.dt.float32
    bf16 = mybir.dt.bfloat16
    n_streams = _N_STREAMS[variant]
    style = _STYLE[variant]
    Bs = B_CORE // n_streams
    wdt = bf16 if style in _BF_STYLES else fp32
    PB = _pb(style)
    out_i8 = style in ("i8", "tanh")

    nc = bass.Bass("TRN2", debug=False)
    d_h0t = nc.dram_tensor("h0t", [H, B_CORE], wdt, kind="ExternalInput").ap()
    d_wt = nc.dram_tensor("wt", [H, 4 * H], wdt, kind="ExternalInput").ap()
    if style not in _TANH_STYLES:
        d_bias = nc.dram_tensor("bias", [H, 4], fp32, kind="ExternalInput").ap()
    d_woutt = nc.dram_tensor("woutt", [H, O], wdt, kind="ExternalInput").ap()
    if style == "biasmm":
        d_onehot = nc.dram_tensor("onehot", [3, 3 * Bs], fp32, kind="ExternalInput").ap()
    if style in _TANH_STYLES:
        d_b4 = nc.dram_tensor("b4", [4, H], bf16, kind="ExternalInput").ap()
        d_onehot4 = nc.dram_tensor("onehot4", [4, 4 * B_CORE], bf16, kind="ExternalInput").ap()
    if style in _BF_STYLES:
        d_ones = nc.dram_tensor("ones", [1, B_CORE], bf16, kind="ExternalInput").ap()
        d_brow = nc.dram_tensor("brow", [1, PB * O], bf16, kind="ExternalInput").ap()
    if out_i8:
        # int8 predictions + per-batch-row absmax (host rescales by max/127)
        d_preds = nc.dram_tensor("preds", [B_CORE, T * O], mybir.dt.int8,
                                 kind="ExternalOutput").ap()
        d_oscale = nc.dram_tensor("oscale", [B_CORE, 1], fp32,
                                  kind="ExternalOutput").ap()
    else:
        out_dt = bf16 if style in _BF_STYLES else fp32
        d_preds = nc.dram_tensor("preds", [B_CORE, T * O], out_dt,
                                 kind="ExternalOutput").ap()

    with tile.TileContext(nc) as tc:
        with (
            tc.tile_pool(name="fixed", bufs=1) as fixed,
            tc.tile_pool(name="state", bufs=2) as state,
            tc.tile_pool(name="acts", bufs=ACTS_BUFS) as acts,
            tc.tile_pool(name="psum", bufs=2, space="PSUM") as psum_pool,
            tc.tile_pool(name="ppsum", bufs=2, space="PSUM") as ppsum_pool,
            tc.tile_pool(name="outp", bufs=1) as outp,
        ):
            consts = {"T": T}
            wt = fixed.tile([H, 4 * H], wdt)
            nc.sync.dma_start(wt[:], d_wt[:])
            if style not in _TANH_STYLES:
                bias = fixed.tile([H, 4], fp32)
                nc.sync.dma_start(bias[:], d_bias[:])
                consts.update(bias=bias)
            woutt = fixed.tile([H, O], wdt)
            nc.sync.dma_start(woutt[:], d_woutt[:])
            consts.update(wt=wt, woutt=woutt)
            if style in _TANH_STYLES:
                b4 = fixed.tile([4, H], bf16)
                nc.sync.dma_start(b4[:], d_b4[:])
                onehot4 = fixed.tile([4, 4 * B_CORE], bf16)
                nc.sync.dma_start(onehot4[:], d_onehot4[:])
                consts.update(b4=b4, onehot4=onehot4)
            if style == "biasmm":
                bstack = fixed.tile([3, H], fp32)
                nc.sync.dma_start(bstack[:], d_bias.rearrange("h g -> g h")[0:3, :])
                onehot = fixed.tile([3, 3 * Bs], fp32)
                nc.sync.dma_start(onehot[:], d_onehot[:])
                consts.update(bstack=bstack, onehot=onehot)
            if style in _BF_STYLES:
                ones = fixed.tile([1, B_CORE], bf16)
                nc.sync.dma_start(ones[:], d_ones[:])
                brow = fixed.tile([1, PB * O], bf16)
                nc.sync.dma_start(brow[:], d_brow[:])
                consts.update(ones=ones, brow=brow)

            pools = (state, acts, psum_pool, ppsum_pool, outp)
            _Stream.gp_t2 = variant == 9
            _Stream.split_o = variant in _TANH_SPLIT_O
            _Stream.psum_bufs = 2 if n_streams <= 2 else 1
            sizes, offs = _bs_list(n_streams)
            streams = [
                _Stream(nc, tc, pools, consts, s, sizes[s], style, off=offs[s])
                for s in range(n_streams)
            ]
            # initial state
            h_dt = bf16 if style in _BF_STYLES else fp32
            h0s = []
            c0s = []
            for s, st in enumerate(streams):
                h0 = state.tile([H, st.Bs], h_dt, tag=f"h{s}", name=f"h0_{s}")
                nc.sync.dma_start(h0[:], d_h0t[:, st.off : st.off + st.Bs])
                # startup stagger: delay stream s's first step by a chain of
                # s*STAGGER self-copies (~220ns each on DVE) so the streams
                # start out of phase and their ACT visits interleave instead
                # of colliding (each stream's period is chain-set, so the
                # initial offset largely persists)
                for _ in range(s * STAGGER):
                    nc.vector.tensor_copy(h0[:], h0[:])
                st.h = h0
                h0s.append(h0)
                if style in _TANH_STYLES:
                    # c' lives in the c-slot of the t4 ping-pong buffers;
                    # step t writes c'_t into t4s[(t+1)%2], so for even T the
                    # final state lands back in t4s[0] (repeat-loop safe;
                    # odd T gets an explicit copy below)
                    nc.vector.memset(st.t4s[0][:, 0 : st.Bs], 0.0)
                else:
                    c0 = state.tile([H, Bs], fp32, tag=f"c{s}", name=f"c0_{s}")
                    nc.vector.memset(c0[:], 0.0)
                    st.c = c0
                    c0s.append(c0)

            def body():
                for t in range(T):
                    for st in streams:
                        st.step(t)

            if repeat > 1:
                with tc.For_i(0, repeat, 1):
                    body()
                    for s, st in enumerate(streams):
                        st.flush_pred()
                        nc.vector.tensor_copy(h0s[s][:], st.h[:])
                        st.h = h0s[s]
                        if style in _TANH_STYLES:
                            if T % 2 == 1:
                                # final c' sits in t4s[1]; move it home
                                nc.vector.tensor_copy(
                                    st.t4s[0][:, 0 : st.Bs],
                                    st.t4s[1][:, 0 : st.Bs],
                                )
                        else:
                            nc.vector.tensor_copy(c0s[s][:], st.c[:])
                            st.c = c0s[s]
            else:
                body()
                for st in streams:
                    st.flush_pred()

            if out_i8:
                AF = mybir.ActivationFunctionType
                i8 = mybir.dt.int8
                for s, st in enumerate(streams):
                    bs, off = st.Bs, st.off
                    rmax = outp.tile([bs, 1], fp32, name=f"rmax{s}")
                    nc.vector.tensor_reduce(
                        rmax[:], st.outbuf[:], mybir.AxisListType.X,
                        mybir.AluOpType.max, apply_absolute_value=True,
                    )
                    rinv = outp.tile([bs, 1], fp32, name=f"rinv{s}")
                    nc.vector.reciprocal(rinv[:], rmax[:])
                    scl = outp.tile([bs, 1], fp32, name=f"scl{s}")
                    nc.vector.tensor_scalar_mul(scl[:], rinv[:], 127.0)
                    qout = outp.tile([bs, T * O], i8, name=f"qout{s}")
                    nc.scalar.activation(qout[:], st.outbuf[:], AF.Copy, scale=scl[:, 0:1])
                    nc.sync.dma_start(d_preds[off : off + bs, :], qout[:])
                    nc.sync.dma_start(d_oscale[off : off + bs, :], scl[:])
            else:
                for s, st in enumerate(streams):
                    nc.sync.dma_start(d_preds[st.off : st.off + st.Bs, :], st.outbuf[:])

    return nc


_RUNNER_CACHE = {}


def _get_runner(nc):
    """Build (once per program) a jitted shard_map callable over the 8 cores.
    run_bass_kernel_spmd rebuilds its jit closure every call, which retraces
    and re-lowers (including BIR serialization) each time — ~1-2.5s of
    client-side overhead per invocation. Caching the jitted callable makes
    repeat invocations cheap.

    The XLA custom call needs its output buffers passed in as donated
    parameters. Uploading fresh zeros for them every call costs a full
    output-sized host->device transfer over the slow axon tunnel, so the
    previous call's device-resident output arrays are recycled as the next
    call's donated seeds (the kernel overwrites every output element, so
    seed contents are irrelevant)."""
    key = id(nc)
    if key in _RUNNER_CACHE:
        return _RUNNER_CACHE[key]

    import jax
    import numpy as np_
    from jax.sharding import Mesh, PartitionSpec
    from jax.experimental.shard_map import shard_map
    import concourse.mybir as mybir
    from concourse.bass2jax import (
        _bass_exec_p,
        install_neuronx_cc_hook,
        partition_id_tensor,
    )

    install_neuronx_cc_hook()

    partition_name = nc.partition_id_tensor.name if nc.partition_id_tensor else None
    in_names = []
    out_names = []
    out_avals = []
    zero_shapes = []
    for alloc in nc.m.functions[0].allocations:
        if not isinstance(alloc, mybir.MemoryLocationSet):
            continue
        name = alloc.memorylocations[0].name
        if alloc.kind == "ExternalInput":
            if name != partition_name:
                in_names.append(name)
        elif alloc.kind == "ExternalOutput":
            shape = tuple(alloc.tensor_shape)
            dtype = mybir.dt.np(alloc.dtype)
            out_names.append(name)
            out_avals.append(jax.core.ShapedArray(shape, dtype))
            zero_shapes.append((shape, dtype))
    n_params = len(in_names)
    n_outs = len(out_names)
    all_in_names = list(in_names) + list(out_names)
    if partition_name is not None:
        all_in_names.append(partition_name)

    def _body(*args):
        operands = list(args)
        if partition_name is not None:
            operands.append(partition_id_tensor())
        outs = _bass_exec_p.bind(
            *operands,
            out_avals=tuple(out_avals),
            in_names=tuple(all_in_names),
            out_names=tuple(out_names),
            lowering_input_output_aliases=(),
            sim_require_finite=True,
            sim_require_nnan=True,
            nc=nc,
        )
        return tuple(outs)

    donate = tuple(range(n_params, n_params + n_outs))
    devices = jax.devices()[:N_CORES]
    mesh = Mesh(np_.asarray(devices), ("core",))
    in_specs = (PartitionSpec("core"),) * (n_params + n_outs)
    out_specs = (PartitionSpec("core"),) * n_outs
    sharded = jax.jit(
        shard_map(_body, mesh=mesh, in_specs=in_specs, out_specs=out_specs, check_rep=False),
        donate_argnums=donate,
        keep_unused=True,
    )

    state = {"seed": None, "host_in": [None] * n_params, "dev_in": [None] * n_params}
    core_sharding = jax.sharding.NamedSharding(mesh, PartitionSpec("core"))

    def _zero_seed():
        # device_put so call #1 sees the same (committed, P("core")-sharded)
        # seed avals as calls #2+ (which recycle device arrays) — avoids a
        # second jit compile on the first warm call
        return [
            jax.device_put(np.zeros((N_CORES * s[0], *s[1:]), d), core_sharding)
            for s, d in zero_shapes
        ]

    def _to_device(concat_in):
        # Device-resident input cache validated by full equality against the
        # last-uploaded host bytes (memcmp speed, ~0.3ms for 1.3MB). In the
        # common repeated-inference case (weights unchanged call to call)
        # this skips the ~1.3MB re-upload over the slow tunnel entirely; any
        # mismatch falls back to a fresh device_put, so results are correct
        # for arbitrary per-call inputs.
        dev = []
        for i, arr in enumerate(concat_in):
            cached_host = state["host_in"][i]
            cached_dev = state["dev_in"][i]
            if (
                cached_dev is not None
                and cached_host is not None
                and cached_host.shape == arr.shape
                and cached_host.dtype == arr.dtype
                and np.array_equal(
                    cached_host.view(np.uint8), arr.view(np.uint8)
                )
            ):
                dev.append(cached_dev)
                continue
            d = jax.device_put(arr, core_sharding)
            state["host_in"][i] = arr
            state["dev_in"][i] = d
            dev.append(d)
        return dev

    def run(in_maps, raw=False, i8_dequant=False):
        per_core = [[np.asarray(m[name]) for name in in_names] for m in in_maps]
        concat_in = [
            np.concatenate([per_core[c][i] for c in range(N_CORES)], axis=0)
            for i in range(n_params)
        ]
        dev_in = _to_device(concat_in)
        seed = state["seed"]
        if seed is None:
            seed = _zero_seed()
        try:
            out_arrs = sharded(*dev_in, *seed)
        except Exception:
            # a failed/interrupted earlier call can leave donated (deleted)
            # arrays in the seed cache — retry once with fresh zeros and
            # uncached inputs
            state["seed"] = None
            state["host_in"] = [None] * n_params
            state["dev_in"] = [None] * n_params
            out_arrs = sharded(*_to_device(concat_in), *_zero_seed())
        state["seed"] = list(out_arrs)
        if i8_dequant:
            return _fetch_dequant(out_arrs)
        hosts = _fetch_concurrent(out_arrs)
        if raw:
            # hand back the global [8*B_CORE, ...] host arrays per output
            # without splitting per core
            return hosts
        return [
            {
                name: hosts[i].reshape(N_CORES, *out_avals[i].shape)[c]
                for i, name in enumerate(out_names)
            }
            for c in range(N_CORES)
        ]

    def _fetch_concurrent(out_arrs):
        hosts = [np.empty(a.shape, a.dtype) for a in out_arrs]
        futs = []
        for host, arr in zip(hosts, out_arrs):
            for sh in arr.addressable_shards:
                def _one(host=host, sh=sh):
                    host[sh.index] = np.asarray(sh.data)
                futs.append(_FETCH_POOL.submit(_one))
        for f in futs:
            f.result()
        return hosts

    def _fetch_dequant(out_arrs):
        # i8 fast path: outputs are (preds int8 [B, T*O], oscale f32 [B, 1]).
        # All fetches launch at once; each preds shard dequantizes into the
        # final f32 buffer inside its fetch thread as soon as both it and
        # the (tiny, fast) scale fetch have landed — the multiply rides
        # along behind the other shards' transfers instead of costing a
        # serial host pass afterwards.
        preds_arr, scale_arr = out_arrs
        out = np.empty(preds_arr.shape, np.float32)
        scale_fut = _FETCH_POOL.submit(
            lambda: (1.0 / np.asarray(scale_arr)).astype(np.float32)
        )

        def _one(sh):
            q = np.asarray(sh.data)  # D2H transfer (dominant cost)
            inv = scale_fut.result()  # usually resolved by now
            np.multiply(q, inv[sh.index[0]], out=out[sh.index], dtype=np.float32)

        futs = [_FETCH_POOL.submit(_one, sh) for sh in preds_arr.addressable_shards]
        for f in futs:
            f.result()
        return out

    _RUNNER_CACHE[key] = run
    return run


def _onehot_input(variant):
    if _STYLE[variant] != "biasmm":
        return None
    Bs = B_CORE // _N_STREAMS[variant]
    oh = np.zeros((3, 3 * Bs), dtype=np.float32)
    for g in range(3):
        oh[g, g * Bs : (g + 1) * Bs] = 1.0
    return oh


def _gate_order(variant):
    # order of gate blocks in the wt layout (reference order is i,f,g,o)
    style = _STYLE[variant]
    if style == "biasmm":
        return [0, 1, 3, 2]
    if style in _TANH_STYLES:
        return [2, 1, 0, 3]  # g, f, i, o
    return [0, 1, 2, 3]


def _onehot4_input(variant):
    """Concatenated per-stream one-hot gate masks: stream s's segment spans
    columns [4*off_s, 4*off_s + 4*Bs_s) with 4 blocks of Bs_s."""
    if _STYLE[variant] not in _TANH_STYLES:
        return None
    sizes, offs = _bs_list(_N_STREAMS[variant])
    oh = np.zeros((4, 4 * B_CORE), dtype=BF16)
    for bs, off in zip(sizes, offs):
        for g in range(4):
            oh[g, 4 * off + g * bs : 4 * off + (g + 1) * bs] = 1.0
    return oh


def _rand_in_maps(rng, variant=None):
    """Random dtype-correct per-core input maps (for benching)."""
    if variant is None:
        variant = VARIANT
    style = _STYLE[variant]
    wdt = BF16 if style in _BF_STYLES else np.float32
    m = {
        "h0t": rng.standard_normal((H, B_CORE)).astype(wdt),
        "wt": (rng.standard_normal((H, 4 * H)) / np.sqrt(H)).astype(wdt),
        "woutt": (rng.standard_normal((H, O)) / np.sqrt(H)).astype(wdt),
    }
    if style not in _TANH_STYLES:
        m["bias"] = (rng.standard_normal((H, 4)) / np.sqrt(H)).astype(np.float32)
    else:
        m["b4"] = (rng.standard_normal((4, H)) / np.sqrt(H)).astype(BF16)
        m["onehot4"] = _onehot4_input(variant)
    if style in _BF_STYLES:
        pb = _pb(style)
        m["ones"] = np.ones((1, B_CORE), dtype=BF16)
        m["brow"] = np.tile(
            (rng.standard_normal(O) / np.sqrt(H)).astype(BF16), pb
        ).reshape(1, pb * O)
    oh = _onehot_input(variant)
    if oh is not None:
        m["onehot"] = oh
    return [dict(m) for _ in range(N_CORES)]


def _get_program(T: int):
    key = (T, VARIANT)
    if key not in _PROGRAM_CACHE:
        _PROGRAM_CACHE[key] = _build_program(T)
    return _PROGRAM_CACHE[key]


def kernel(
    context_seq,
    W_ih,
    W_hh,
    b_ih,
    b_hh,
    W_out,
    b_out,
    prediction_len,
):
    T = int(prediction_len)
    context_seq = np.asarray(context_seq)
    W_ih = np.asarray(W_ih, dtype=np.float32)
    W_hh = np.asarray(W_hh, dtype=np.float32)
    b_ih = np.asarray(b_ih, dtype=np.float32)
    b_hh = np.asarray(b_hh, dtype=np.float32)
    W_out = np.asarray(W_out, dtype=np.float32)
    b_out = np.asarray(b_out, dtype=np.float32)

    B = context_seq.shape[0]
    assert B == B_TOTAL and context_seq.shape[2] == H

    style = _STYLE[VARIANT]
    wdt = BF16 if style in _BF_STYLES else np.float32

    # Host-side prep: only the last timestep of context_seq is used.
    h0 = np.asarray(context_seq[:, -1, :], dtype=np.float32)  # [B, H]
    W = W_ih + W_hh  # [4H, H]
    b = b_ih + b_hh  # [4H]
    order = _gate_order(VARIANT)
    Wb = W.reshape(4, H, H)[order]
    bb = b.reshape(4, H)[order]
    if style in _TANH_STYLES:
        # all-tanh form: sigmoid(z) = 0.5 + 0.5*tanh(z/2) for gates f,i,o and
        # tanh(z) for g, with h tracked as h' = 2h. The tanh argument for a
        # sigmoid gate is z/2 = h'@(W/4).T + b/2; for g it's h'@(W/2).T + b.
        # Block order here is (g, f, i, o) per _gate_order.
        wscale = np.array([0.5, 0.25, 0.25, 0.25], dtype=np.float32)
        bscale = np.array([1.0, 0.5, 0.5, 0.5], dtype=np.float32)
        Wb = Wb * wscale[:, None, None]
        bb = bb * bscale[:, None]
        h0 = 2.0 * h0
        W_out = 0.5 * W_out
    wt = np.ascontiguousarray(Wb.reshape(4 * H, H).T).astype(wdt)  # [H, 4H]
    bias_cols = np.ascontiguousarray(bb.T)  # [H, 4] f32
    woutt = np.ascontiguousarray(W_out.T).astype(wdt)  # [H, O]

    nc = _get_program(T)

    PB = _pb(style)
    ones = np.ones((1, B_CORE), dtype=BF16)
    brow = np.tile(b_out.astype(BF16), PB).reshape(1, PB * O)

    in_maps = []
    for c in range(N_CORES):
        sh = h0[c * B_CORE : (c + 1) * B_CORE]  # [B_CORE, H]
        m = {
            "h0t": np.ascontiguousarray(sh.T).astype(wdt),  # [H, B_CORE]
            "wt": wt,
            "woutt": woutt,
        }
        if style in _TANH_STYLES:
            m["b4"] = np.ascontiguousarray(bb).astype(BF16)  # [4, H]
            m["onehot4"] = _onehot4_input(VARIANT)
        else:
            m["bias"] = bias_cols
        if style in _BF_STYLES:
            m["ones"] = ones
            m["brow"] = brow
        oh = _onehot_input(VARIANT)
        if oh is not None:
            m["onehot"] = oh
        in_maps.append(m)

    run = _get_runner(nc)

    if style == "bf16":
        # b_out already added on device; single astype pass gathers+converts
        out = run(in_maps, raw=True)[0]  # [B_TOTAL, T*O] bf16
        return out.astype(np.float32).reshape(B_TOTAL, T, O)

    if style in ("i8", "tanh"):
        # b_out added on device; predictions arrive int8 with a per-batch-row
        # scale — dequantized shard-by-shard inside the fetch threads
        out = run(in_maps, i8_dequant=True)  # [B_TOTAL, T*O] f32
        return out.reshape(B_TOTAL, T, O)

    results = run(in_maps)
    out = np.empty((B_TOTAL, T, O), dtype=np.float32)
    for c in range(N_CORES):
        out[c * B_CORE : (c + 1) * B_CORE] = results[c]["preds"].reshape(B_CORE, T, O)
    out += b_out  # broadcast over [B, T, O]
    return out

